# revision 35
# baseline (speedup 1.0000x reference)
"""Multi-head attention (B=4, S=1500, D=1024, H=16) on 8 TRN2 NeuronCores.

Sharding: (batch, head-half) -> core c = 2*b + h; each core computes the full
attention for batch b, heads h*8..h*8+7, plus its partial contribution to the
output projection (contraction over its 512 features). Host sums the two
partials per batch, adds bo + Wo@bv (the v-bias term commutes through the
output projection), and stacks.

Numerics (end-to-end max-rel err ~1.5e-2, dominated by one fp8 cast of q):
  q/k/v projections run on the PE in fp8e4 + DoubleRow perf mode (0.5
  cycles/row, 256 contraction rows/instr) using a 3-term residual split
  x@W ~= x8@W8 + x8@Wr + xr@W8 (x8/W8 fp8 casts, xr/Wr fp8 casts of the
  cast residuals; the dropped xr@Wr term is ~0.07%). Wq/Wk/Wv are
  host-scaled by 32 so their fp8 mantissas are in range; the scale is
  compensated in the exp scale (q and k both carry 32x) and in the v
  staging (x 1/32).
  scores S^T[k,q] per head: one DoubleRow matmul with lhsT groups =
  (k_hi, k_lo) fp8 (exact k) and rhs groups = (q8, q8) (one fp8 cast of q:
  the only significant error). exp on ACT (the critical engine, ~138us)
  -> P^T in fp16.
  U: WVn[q, 65] += pt[k, qchunk].T @ v_aug[k, 65] in fp16 (full output
  lanes, N=65). The 12 q-tile accumulators pack into 2 PSUM banks via DVE
  memset + start=False accumulation. v's 65th ones-column gives softmax
  denominators. normalize: per-partition reciprocal + TSP -> WV[q,f] fp16;
  WV -> wvT via XBAR DMA-transpose (no PE/DVE cost). out-projection fp16.

Emission is software-pipelined around the ACT exp cadence: head-pair tensor
t0 is projected up front, t1-t3 stream through spare slots of the first
windows; v-projection borrows the uacc PSUM banks during head 0's window;
U(h) runs during head h+1's window; PSUM = 2x [128,1536] (S^T + projection
accumulators, rotating) + 2x [128,512] accumulator banks.
"""

import os
import numpy as np

N_STATE = 1024
B = 4
S = 1500
F = 512            # features per core (8 heads x 64)
NKT = 12           # seq k-tiles of 128, last = 92
KPAD = 1536
VBLK = 520         # 8 heads * 65 cols (64 d + ones) per seq tile in v_sb
SLOTW = 80         # uacc accumulator slot stride (f32 cols)
QCH = [(0, 512), (512, 512), (1024, 476)]
SCALE = 0.125      # 1/sqrt(64)
WSCALE = 32.0      # host scaling of Wq/Wk/Wv before fp8 cast
EXP_SCALE = SCALE / (WSCALE * WSCALE)
NCORES = 8

_CACHE = {}
LAST_RESULTS = None


def _build():
    import concourse.mybir as mybir
    import concourse.tile as tile
    from concourse import bacc

    f32 = mybir.dt.float32
    fp16 = mybir.dt.float16
    fp8 = mybir.dt.float8e4
    Exp = mybir.ActivationFunctionType.Exp
    Copy = mybir.ActivationFunctionType.Copy
    DR = mybir.MatmulPerfMode.DoubleRow

    nc = bacc.Bacc("TRN2", target_bir_lowering=False, debug=False,
                   num_devices=NCORES)

    x8d = nc.dram_tensor("x8d", [128, 4, 2, KPAD], fp8,
                         kind="ExternalInput").ap()
    xrd = nc.dram_tensor("xrd", [128, 4, 2, KPAD], fp8,
                         kind="ExternalInput").ap()
    wq8 = nc.dram_tensor("wq8", [4, 128, 4, 2, 128], fp8,
                         kind="ExternalInput").ap()
    wqr = nc.dram_tensor("wqr", [4, 128, 4, 2, 128], fp8,
                         kind="ExternalInput").ap()
    wk8 = nc.dram_tensor("wk8", [4, 128, 4, 2, 128], fp8,
                         kind="ExternalInput").ap()
    wkr = nc.dram_tensor("wkr", [4, 128, 4, 2, 128], fp8,
                         kind="ExternalInput").ap()
    wv8 = nc.dram_tensor("wv8", [128, 4, 2, F], fp8, kind="ExternalInput").ap()
    wvr = nc.dram_tensor("wvr", [128, 4, 2, F], fp8, kind="ExternalInput").ap()
    wod = nc.dram_tensor("wod", [128, 4, N_STATE], fp16,
                         kind="ExternalInput").ap()
    bqh = nc.dram_tensor("bqh", [128, 4], f32, kind="ExternalInput").ap()
    y = nc.dram_tensor("y", [S, N_STATE], fp16, kind="ExternalOutput").ap()

    mm = nc.tensor.matmul

    with tile.TileContext(nc) as tc:
        with (
            tc.tile_pool(name="sb", bufs=1) as sb,
            tc.tile_pool(name="sbw", bufs=16) as sbw,
            tc.tile_pool(name="ptp", bufs=24) as ptp,
            tc.tile_pool(name="sm", bufs=3) as smp,
            tc.tile_pool(name="ysp", bufs=4) as ysp,
            tc.tile_pool(name="ps3", bufs=2, space="PSUM") as ps3,
            tc.tile_pool(name="psu", bufs=1, space="PSUM") as psu,
        ):
            # ---------------- persistent SBUF ----------------
            x8_sb = sb.tile([128, 4, 2, KPAD], fp8, name="x8_sb", tag="x8")
            xr_sb = sb.tile([128, 4, 2, KPAD], fp8, name="xr_sb", tag="xr")
            # per head-pair tensor t: partitions p = head 2t + p//64,
            # d = p%64; q groups both hold q8, k groups hold (k_hi, k_lo)
            qdr = [sb.tile([128, S], fp8, name=f"qdr{t}", tag=f"qdr{t}")
                   for t in range(4)]
            kdr = [sb.tile([128, 2, KPAD], fp8, name=f"kdr{t}", tag=f"kdr{t}")
                   for t in range(4)]
            v_sb = sb.tile([128, NKT * VBLK], fp16, name="v_sb", tag="v")
            wv8_sb = sb.tile([128, 4, 2, F], fp8, name="wv8_sb", tag="wv8")
            wvr_sb = sb.tile([128, 4, 2, F], fp8, name="wvr_sb", tag="wvr")
            wo_sb = sb.tile([128, 4, N_STATE], fp16, name="wo_sb", tag="wo")
            WV_sb = sb.tile([128, NKT, F], fp16, name="WV_sb", tag="WV")
            wvT_sb = sb.tile([128, 4, KPAD], fp16, name="wvT_sb", tag="wvT")
            bq_sb = sb.tile([128, 4], f32, name="bq_sb", tag="bq")
            zero_col = sb.tile([128, 1], f32, name="zero_col", tag="zc")
            i32_col = sb.tile([128, 1], f32, name="i32_col", tag="i32")

            # uacc banks double as the v-projection / U accumulators
            uaccA = psu.tile([128, 512], f32, name="uaccA", tag="uaccA")
            uaccB = psu.tile([128, 512], f32, name="uaccB", tag="uaccB")

            # ---------------- input DMAs ----------------
            def load_w(dram, t):
                wsl = sbw.tile([128, 4, 2, 128], fp8, name="wsl", tag="wsl")
                nc.sync.dma_start(out=wsl[:], in_=dram[t])
                return wsl

            wslq8, wslqr = [load_w(wq8, 0)], [load_w(wqr, 0)]
            nc.sync.dma_start(out=x8_sb[:, :, :, 0:512],
                              in_=x8d[:, :, :, 0:512])
            nc.sync.dma_start(out=xr_sb[:, :, :, 0:512],
                              in_=xrd[:, :, :, 0:512])
            wslk8, wslkr = [load_w(wk8, 0)], [load_w(wkr, 0)]
            nc.sync.dma_start(out=bq_sb[:], in_=bqh)
            nc.vector.memset(zero_col[:], 0.0)
            nc.vector.memset(i32_col[:], 1.0 / WSCALE)
            nc.vector.memset(WV_sb[64:128, NKT - 1, :], 0.0)
            # v ones-columns and k pad columns via memset (no DMA)
            nc.vector.memset(
                v_sb[:].rearrange("p (t h c) -> p t h c",
                                  t=NKT, h=8)[:, :, :, 64:65], 1.0)
            for t in range(4):
                nc.vector.memset(kdr[t][:, :, S:KPAD], 0.0)
            # warm the ACT exp table off the critical path
            warm = smp.tile([128, 1], f32, name="warm", tag="warm")
            nc.scalar.activation(warm[:], zero_col[:], Exp, scale=1.0)
            nc.sync.dma_start(out=x8_sb[:, :, :, 512:KPAD],
                              in_=x8d[:, :, :, 512:KPAD])
            nc.sync.dma_start(out=xr_sb[:, :, :, 512:KPAD],
                              in_=xrd[:, :, :, 512:KPAD])
            nc.sync.dma_start(out=wv8_sb[:], in_=wv8)
            nc.sync.dma_start(out=wvr_sb[:], in_=wvr)
            # head-pair tensors t1-3: one DMA per weight tensor
            def load_w3(dram):
                w3 = sbw.tile([128, 3, 4, 2, 128], fp8, name="w3", tag="w3",
                              bufs=4)
                nc.sync.dma_start(
                    out=w3[:], in_=dram[1:4].rearrange("t p g i m -> p t g i m"))
                return [w3[:, t - 1] for t in range(1, 4)]
            wslq8 += load_w3(wq8)
            wslqr += load_w3(wqr)
            wslk8 += load_w3(wk8)
            wslkr += load_w3(wkr)
            nc.sync.dma_start(out=wo_sb[:], in_=wod)

            # ---------------- projections (fp8 DR, 3-term residual) -------
            def proj_chunk(w8sl, wrsl, pacc, q0, qn):
                pairs = [(w8sl, x8_sb), (wrsl, x8_sb), (w8sl, xr_sb)]
                for ti, (wsl, xs) in enumerate(pairs):
                    for g in range(4):
                        mm(out=pacc[:, q0:q0 + qn],
                           lhsT=wsl[:, g, :, :],
                           rhs=xs[:, g, :, q0:q0 + qn],
                           start=(ti == 0 and g == 0),
                           stop=(ti == 2 and g == 3), perf_mode=DR)

            def q_stage(pacc, t, q0, qn):
                nc.vector.tensor_scalar_add(
                    out=qdr[t][:, q0:q0 + qn], in0=pacc[:, q0:q0 + qn],
                    scalar1=bq_sb[:, t:t + 1])

            def k_stage(pacc, t, q0, qn):
                nc.vector.tensor_scalar_add(
                    out=kdr[t][:, 0, q0:q0 + qn], in0=pacc[:, q0:q0 + qn],
                    scalar1=zero_col[:, 0:1])
                nc.vector.tensor_sub(kdr[t][:, 1, q0:q0 + qn],
                                     pacc[:, q0:q0 + qn],
                                     kdr[t][:, 0, q0:q0 + qn])

            # one q-chunk per unit so each borrows a big3 slot only briefly
            # (allocate, accumulate 12 DR matmuls, stage)
            def late_proj_unit(qk, t, ch):
                pacc = ps3.tile([128, 1536], f32, name="pacc", tag="big3")
                q0, qn = QCH[ch]
                if qk == "q":
                    proj_chunk(wslq8[t], wslqr[t], pacc, q0, qn)
                    q_stage(pacc, t, q0, qn)
                else:
                    proj_chunk(wslk8[t], wslkr[t], pacc, q0, qn)
                    k_stage(pacc, t, q0, qn)

            # head-pair t0 up front: all q chunks + k chunk 0 (enough for
            # S^T(h0, kt0-3)); k chunks 1/2 follow as window-0 fillers
            late_proj_unit("q", 0, 0)
            late_proj_unit("k", 0, 0)
            late_proj_unit("q", 0, 1)
            late_proj_unit("q", 0, 2)

            # ---------------- attention building blocks ----------------
            def st_exp(h, kt):
                t, r = h // 2, 64 * (h % 2)
                stp = ps3.tile([128, 1536], f32, name="stp", tag="big3")
                for q0, qn in QCH:
                    mm(out=stp[:, q0:q0 + qn],
                       lhsT=kdr[t][r:r + 64, :, kt * 128:(kt + 1) * 128],
                       rhs=qdr[t][r:r + 64, q0:q0 + qn][:, None, :
                                                        ].to_broadcast(
                           (64, 2, qn)),
                       start=True, stop=True, perf_mode=DR)
                pt = ptp.tile([128, 1536], fp16, name="pt", tag="pt")
                nc.scalar.activation(pt[:, 0:S], stp[:, 0:S], Exp,
                                     scale=EXP_SCALE)
                return pt

            def u_emit(h, kt, pt, accA=None, accB=None):
                accA = uaccA if accA is None else accA
                accB = uaccB if accB is None else accB
                kn = min(128, S - kt * 128)
                vcol = kt * VBLK + h * 65
                for qt in range(NKT):
                    qn = min(128, S - qt * 128)
                    acc = accA if qt < 6 else accB
                    slot = (qt % 6) * SLOTW
                    mm(out=acc[0:qn, slot:slot + 65],
                       lhsT=pt[0:kn, qt * 128:qt * 128 + qn],
                       rhs=v_sb[0:kn, vcol:vcol + 65],
                       start=False, stop=False, skip_group_check=True)

            def v_emit(sq):
                sn = min(128, S - sq * 128)
                acc = uaccA if sq % 2 == 0 else uaccB
                first, last = True, False
                for ti, (xs, ws) in enumerate(
                        [(x8_sb, wv8_sb), (x8_sb, wvr_sb), (xr_sb, wv8_sb)]):
                    for g in range(4):
                        mm(out=acc[0:sn, 0:512],
                           lhsT=xs[:, g, :, sq * 128:sq * 128 + sn],
                           rhs=ws[:, g, :, :],
                           start=(ti == 0 and g == 0),
                           stop=(ti == 2 and g == 3), perf_mode=DR)
                nc.vector.tensor_scalar_mul(
                    out=v_sb[0:sn, sq * VBLK:(sq + 1) * VBLK].rearrange(
                        "p (h c) -> p h c", h=8)[:, :, 0:64],
                    in0=acc[0:sn, 0:512].rearrange("p (h c) -> p h c", h=8),
                    scalar1=i32_col[0:sn, 0:1])

            def memsets():
                nc.vector.memset(uaccA[:], 0.0)
                nc.vector.memset(uaccB[:], 0.0)

            def norm_emit(h, accA=None, accB=None):
                accA = uaccA if accA is None else accA
                accB = uaccB if accB is None else accB
                rc = smp.tile([128, 12], f32, name="rc", tag="rc")
                nc.vector.reciprocal(
                    rc[:, 0:6].rearrange("p (s c) -> p s c", c=1),
                    accA[:, 0:6 * SLOTW].rearrange(
                        "p (s c) -> p s c", s=6)[:, :, 64:65])
                nc.vector.reciprocal(
                    rc[:, 6:11].rearrange("p (s c) -> p s c", c=1),
                    accB[:, 0:5 * SLOTW].rearrange(
                        "p (s c) -> p s c", s=5)[:, :, 64:65])
                nc.vector.reciprocal(rc[0:92, 11:12],
                                     accB[0:92, 5 * SLOTW + 64:5 * SLOTW + 65])
                for qt in range(NKT):
                    sn = min(128, S - qt * 128)
                    acc = accA if qt < 6 else accB
                    slot = (qt % 6) * SLOTW
                    nc.vector.tensor_scalar_mul(
                        out=WV_sb[0:sn, qt, h * 64:(h + 1) * 64],
                        in0=acc[0:sn, slot:slot + 64],
                        scalar1=rc[0:sn, qt:qt + 1])

            def transp_emit(p):
                for qt in range(NKT):
                    nc.sync.dma_start(
                        out=wvT_sb[:, p, qt * 128:(qt + 1) * 128],
                        in_=WV_sb[:, qt, p * 128:(p + 1) * 128],
                        transpose=True)

            # one q-chunk per unit so each borrows a big3 slot only briefly
            # (allocate, accumulate 12 DR matmuls, stage)
            def late_proj_unit(qk, t, ch):
                pacc = ps3.tile([128, 1536], f32, name="pacc", tag="big3")
                q0, qn = QCH[ch]
                if qk == "q":
                    proj_chunk(wslq8[t], wslqr[t], pacc, q0, qn)
                    q_stage(pacc, t, q0, qn)
                else:
                    proj_chunk(wslk8[t], wslkr[t], pacc, q0, qn)
                    k_stage(pacc, t, q0, qn)

            # head-pair t0 up front: all q chunks + k chunk 0 (enough for
            # S^T(h0, kt0-3)); k chunks 1/2 follow as window-0 fillers
            late_proj_unit("q", 0, 0)
            late_proj_unit("k", 0, 0)
            late_proj_unit("q", 0, 1)
            late_proj_unit("q", 0, 2)

            # ---------------- attention building blocks ----------------
            def st_exp(h, kt):
                t, r = h // 2, 64 * (h % 2)
                stp = ps3.tile([128, 1536], f32, name="stp", tag="big3")
                for q0, qn in QCH:
                    mm(out=stp[:, q0:q0 + qn],
                       lhsT=kdr[t][r:r + 64, :, kt * 128:(kt + 1) * 128],
                       rhs=qdr[t][r:r + 64, q0:q0 + qn][:, None, :
                                                        ].to_broadcast(
                           (64, 2, qn)),
                       start=True, stop=True, perf_mode=DR)
                pt = ptp.tile([128, 1536], fp16, name="pt", tag="pt")
                nc.scalar.activation(pt[:, 0:S], stp[:, 0:S], Exp,
                                     scale=EXP_SCALE)
                return pt

            def u_emit(h, kt, pt, accA=None, accB=None):
                accA = uaccA if accA is None else accA
                accB = uaccB if accB is None else accB
                kn = min(128, S - kt * 128)
                vcol = kt * VBLK + h * 65
                for qt in range(NKT):
                    qn = min(128, S - qt * 128)
                    acc = accA if qt < 6 else accB
                    slot = (qt % 6) * SLOTW
                    mm(out=acc[0:qn, slot:slot + 65],
                       lhsT=pt[0:kn, qt * 128:qt * 128 + qn],
                       rhs=v_sb[0:kn, vcol:vcol + 65],
                       start=False, stop=False, skip_group_check=True)

            def v_emit(sq):
                sn = min(128, S - sq * 128)
                acc = uaccA if sq % 2 == 0 else uaccB
                first, last = True, False
                for ti, (xs, ws) in enumerate(
                        [(x8_sb, wv8_sb), (x8_sb, wvr_sb), (xr_sb, wv8_sb)]):
                    for g in range(4):
                        mm(out=acc[0:sn, 0:512],
                           lhsT=xs[:, g, :, sq * 128:sq * 128 + sn],
                           rhs=ws[:, g, :, :],
                           start=(ti == 0 and g == 0),
                           stop=(ti == 2 and g == 3), perf_mode=DR)
                nc.vector.tensor_scalar_mul(
                    out=v_sb[0:sn, sq * VBLK:(sq + 1) * VBLK].rearrange(
                        "p (h c) -> p h c", h=8)[:, :, 0:64],
                    in0=acc[0:sn, 0:512].rearrange("p (h c) -> p h c", h=8),
                    scalar1=i32_col[0:sn, 0:1])

            def memsets():
                nc.vector.memset(uaccA[:], 0.0)
                nc.vector.memset(uaccB[:], 0.0)

            def norm_emit(h, accA=None, accB=None):
                accA = uaccA if accA is None else accA
                accB = uaccB if accB is None else accB
                rc = smp.tile([128, 12], f32, name="rc", tag="rc")
                nc.vector.reciprocal(
                    rc[:, 0:6].rearrange("p (s c) -> p s c", c=1),
                    accA[:, 0:6 * SLOTW].rearrange(
                        "p (s c) -> p s c", s=6)[:, :, 64:65])
                nc.vector.reciprocal(
                    rc[:, 6:11].rearrange("p (s c) -> p s c", c=1),
                    accB[:, 0:5 * SLOTW].rearrange(
                        "p (s c) -> p s c", s=5)[:, :, 64:65])
                nc.vector.reciprocal(rc[0:92, 11:12],
                                     accB[0:92, 5 * SLOTW + 64:5 * SLOTW + 65])
                for qt in range(NKT):
                    sn = min(128, S - qt * 128)
                    acc = accA if qt < 6 else accB
                    slot = (qt % 6) * SLOTW
                    nc.vector.tensor_scalar_mul(
                        out=WV_sb[0:sn, qt, h * 64:(h + 1) * 64],
                        in0=acc[0:sn, slot:slot + 64],
                        scalar1=rc[0:sn, qt:qt + 1])

            def transp_emit(p):
                for qt in range(NKT):
                    nc.sync.dma_start(
                        out=wvT_sb[:, p, qt * 128:(qt + 1) * 128],
                        in_=WV_sb[:, qt, p * 128:(p + 1) * 128],
                        transpose=True)

            # out-projection split in two parts: fc0-2 accumulate early, fc3
            # joins once the pair-3 transposes land; both halves of a q-tile
            # stage into one yt so y goes out as a single DMA per q-tile
            def out_a(sq, ch):
                sn = min(128, S - sq * 128)
                py = ps3.tile([128, 1536], f32, name="py", tag="big3")
                for fc in range(3):
                    mm(out=py[0:sn, 0:512],
                       lhsT=wvT_sb[:, fc, sq * 128:sq * 128 + sn],
                       rhs=wo_sb[:, fc, ch * 512:(ch + 1) * 512],
                       start=(fc == 0), stop=False, skip_group_check=True)
                return py

            def out_b(sq, ch, py, yt):
                sn = min(128, S - sq * 128)
                mm(out=py[0:sn, 0:512],
                   lhsT=wvT_sb[:, 3, sq * 128:sq * 128 + sn],
                   rhs=wo_sb[:, 3, ch * 512:(ch + 1) * 512],
                   start=False, stop=True, skip_group_check=True)
                nc.vector.tensor_scalar_add(out=yt[0:sn, ch * 512:(ch + 1) * 512],
                                            in0=py[0:sn, 0:512],
                                            scalar1=zero_col[0:sn, 0:1])
                if ch == 1:
                    nc.sync.dma_start(out=y[sq * 128:sq * 128 + sn, :],
                                      in_=yt[0:sn, :])

            # ---------------- pipelined attention ----------------
            # window h, slot kt: S^T/exp(h, kt), then the scheduled fillers
            FILL = {}

            def add(h, kt, fn):
                FILL.setdefault((h, kt), []).append(fn)

            add(0, 0, lambda: late_proj_unit("k", 0, 1))
            add(0, 1, lambda: late_proj_unit("k", 0, 2))
            for kt in range(NKT):
                add(0, kt, lambda sq=kt: v_emit(sq))
            add(1, 0, memsets)
            for kt in range(1, NKT):
                add(1, kt, lambda k=kt - 1: u_emit(0, k, pts.pop((0, k))))
            add(1, 11, lambda: u_emit(0, 11, pts.pop((0, 11))))
            # t1 projections during window 1 (needed by window 2)
            for ci, kt in enumerate((2, 4, 6)):
                add(1, kt, lambda c=ci: late_proj_unit("q", 1, c))
            for ci, kt in enumerate((3, 5, 7)):
                add(1, kt, lambda c=ci: late_proj_unit("k", 1, c))
            for h in range(2, 8):
                add(h, 0, lambda hh=h - 2: norm_emit(hh))
                add(h, 1, memsets)
                add(h, 1, lambda hh=h - 1: u_emit(hh, 0, pts.pop((hh, 0))))
                for kt in range(2, NKT):
                    add(h, kt,
                        lambda hh=h - 1, k=kt - 1: u_emit(hh, k,
                                                          pts.pop((hh, k))))
                add(h, 11, lambda hh=h - 1: u_emit(hh, 11,
                                                   pts.pop((hh, 11))))
            # t2 over windows 2-3, t3 over windows 4-5 (first uses: w4 / w6)
            for ci, (hh, kt) in enumerate(((2, 3), (2, 6), (2, 9))):
                add(hh, kt, lambda c=ci: late_proj_unit("q", 2, c))
            for ci, (hh, kt) in enumerate(((3, 3), (3, 6), (3, 9))):
                add(hh, kt, lambda c=ci: late_proj_unit("k", 2, c))
            for ci, (hh, kt) in enumerate(((4, 3), (4, 6), (4, 9))):
                add(hh, kt, lambda c=ci: late_proj_unit("q", 3, c))
            for ci, (hh, kt) in enumerate(((5, 3), (5, 6), (5, 9))):
                add(hh, kt, lambda c=ci: late_proj_unit("k", 3, c))
            add(4, 1, lambda: transp_emit(0))
            add(6, 1, lambda: transp_emit(1))
            add(7, 1, lambda: transp_emit(2))

            pts = {}
            for h in range(8):
                for kt in range(NKT):
                    pts[(h, kt)] = st_exp(h, kt)
                    for fn in FILL.get((h, kt), ()):
                        fn()

            # ---------------- tail ----------------
            # head 7's U accumulates into a spare big3 slot (two clean
            # 512-col bank regions) so norm(6) can drain the uacc banks in
            # parallel; U(7) runs qt-outer with per-qt normalize + transpose
            # so the pair-3 transposes trickle out as early as possible
            u2 = ps3.tile([128, 1536], f32, name="u2", tag="big3")
            nc.vector.memset(u2[:, 0:1024], 0.0)
            u2A, u2B = u2[:, 0:512], u2[:, 512:1024]
            norm_emit(6)
            rc7 = smp.tile([128, 12], f32, name="rc7", tag="rc")
            for qt in range(NKT):
                qn = min(128, S - qt * 128)
                acc = u2A if qt < 6 else u2B
                slot = (qt % 6) * SLOTW
                for kt in range(NKT):
                    kn = min(128, S - kt * 128)
                    pt = pts[(7, kt)]
                    mm(out=acc[0:qn, slot:slot + 65],
                       lhsT=pt[0:kn, qt * 128:qt * 128 + qn],
                       rhs=v_sb[0:kn, kt * VBLK + 7 * 65:kt * VBLK + 7 * 65 + 65],
                       start=False, stop=False, skip_group_check=True)
                nc.vector.reciprocal(rc7[0:qn, qt:qt + 1],
                                     acc[0:qn, slot + 64:slot + 65])
                nc.vector.tensor_scalar_mul(
                    out=WV_sb[0:qn, qt, 7 * 64:8 * 64],
                    in0=acc[0:qn, slot:slot + 64],
                    scalar1=rc7[0:qn, qt:qt + 1])
                nc.sync.dma_start(
                    out=wvT_sb[:, 3, qt * 128:(qt + 1) * 128],
                    in_=WV_sb[:, qt, 384:512],
                    transpose=True)
            for kt in range(NKT):
                pts.pop((7, kt))
            # out-projection with a 4-deep psum rotation (2 big3 slots +
            # the two freed uacc banks); fc0-2 accumulate first, fc3 joins
            # when the pair-3 transpose for that q-tile lands
            seq = [(sq, ch) for sq in range(NKT) for ch in range(2)]
            pys = {}
            yts = {}
            LAG = 3

            def out_fin(s2, c2):
                sn2 = min(128, S - s2 * 128)
                reg = pys.pop((s2, c2))
                mm(out=reg[0:sn2, 0:512],
                   lhsT=wvT_sb[:, 3, s2 * 128:s2 * 128 + sn2],
                   rhs=wo_sb[:, 3, c2 * 512:(c2 + 1) * 512],
                   start=False, stop=True, skip_group_check=True)
                nc.vector.tensor_scalar_add(
                    out=yts[s2][0:sn2, c2 * 512:(c2 + 1) * 512],
                    in0=reg[0:sn2, 0:512], scalar1=zero_col[0:sn2, 0:1])
                if c2 == 1:
                    nc.sync.dma_start(out=y[s2 * 128:s2 * 128 + sn2, :],
                                      in_=yts[s2][0:sn2, :])

            for i, (sq, ch) in enumerate(seq):
                yts.setdefault(sq, ysp.tile([128, 1024], fp16, name="yt",
                                            tag="yt"))
                sn = min(128, S - sq * 128)
                if i % 4 < 2:
                    reg = ps3.tile([128, 1536], f32, name="py", tag="big3")
                else:
                    reg = uaccA if i % 4 == 2 else uaccB
                for fc in range(3):
                    mm(out=reg[0:sn, 0:512],
                       lhsT=wvT_sb[:, fc, sq * 128:sq * 128 + sn],
                       rhs=wo_sb[:, fc, ch * 512:(ch + 1) * 512],
                       start=(fc == 0), stop=False, skip_group_check=True)
                pys[(sq, ch)] = reg
                if i >= LAG:
                    out_fin(*seq[i - LAG])
            for j in range(len(seq) - LAG, len(seq)):
                out_fin(*seq[j])

    nc.compile()
    return nc


def get_nc():
    if "nc" not in _CACHE:
        _CACHE["nc"] = _build()
    return _CACHE["nc"]


def make_in_maps(x, Wq, bq, Wk, Wv, bv, Wo, bo):
    import ml_dtypes
    e4 = ml_dtypes.float8_e4m3fn
    f16 = np.float16

    x = np.asarray(x, dtype=np.float32)
    Wq = np.asarray(Wq, dtype=np.float32)
    Wk = np.asarray(Wk, dtype=np.float32)
    Wv = np.asarray(Wv, dtype=np.float32)
    Wo = np.asarray(Wo, dtype=np.float32)
    bq = np.asarray(bq, dtype=np.float32)

    def wsplit(Wc):
        # Wc: [512 feat, 1024 state] scaled; -> (hi, lo) each
        # [4, 128(p), 4(g), 2(i), 128(m)] fp8 with W[128t+m, (2g+i)*128+p]
        W8 = Wc.astype(e4)
        Wr = (Wc - W8.astype(np.float32)).astype(e4)
        out = []
        for Wx in (W8, Wr):
            a = Wx.T.reshape(4, 2, 128, 4, 128)     # [g, i, p, t, m]
            out.append(np.ascontiguousarray(a.transpose(3, 2, 0, 1, 4)))
        return out

    def vsplit(Wc):
        # -> [128(p), 4(g), 2(i), 512(f)] fp8
        W8 = Wc.astype(e4)
        Wr = (Wc - W8.astype(np.float32)).astype(e4)
        out = []
        for Wx in (W8, Wr):
            a = Wx.T.reshape(4, 2, 128, F)          # [g, i, p, f]
            out.append(np.ascontiguousarray(a.transpose(2, 0, 1, 3)))
        return out

    in_maps = []
    for c in range(NCORES):
        b, h2 = divmod(c, 2)
        sl = slice(h2 * F, (h2 + 1) * F)
        xT = np.zeros((N_STATE, KPAD), dtype=np.float32)
        xT[:, 0:S] = x[b].T
        x8f = xT.astype(e4)
        xrf = (xT - x8f.astype(np.float32)).astype(e4)
        x8 = x8f.reshape(4, 2, 128, KPAD).transpose(2, 0, 1, 3)
        xr = xrf.reshape(4, 2, 128, KPAD).transpose(2, 0, 1, 3)
        wq8_, wqr_ = wsplit(Wq[sl] * WSCALE)
        wk8_, wkr_ = wsplit(Wk[sl] * WSCALE)
        wv8_, wvr_ = vsplit(Wv[sl] * WSCALE)
        wod_ = Wo[:, sl].T.reshape(4, 128, N_STATE).transpose(1, 0, 2)
        # feature f = 128t + p  ->  bqh[p, t]
        bqh_ = (WSCALE * bq[sl]).reshape(4, 128).T
        in_maps.append(dict(
            x8d=np.ascontiguousarray(x8), xrd=np.ascontiguousarray(xr),
            wq8=wq8_, wqr=wqr_, wk8=wk8_, wkr=wkr_,
            wv8=wv8_, wvr=wvr_,
            wod=np.ascontiguousarray(wod_.astype(f16)),
            bqh=np.ascontiguousarray(bqh_, dtype=np.float32),
        ))
    return in_maps


def kernel(x, Wq, bq, Wk, Wv, bv, Wo, bo):
    global LAST_RESULTS
    from concourse.bass_utils import run_bass_kernel_spmd

    try:
        import antenv.axon_hooks  # noqa: F401
    except ImportError:
        os.environ["BASS_NEVER_TRACE"] = "1"

    nc = get_nc()
    in_maps = make_in_maps(x, Wq, bq, Wk, Wv, bv, Wo, bo)
    res = run_bass_kernel_spmd(nc, in_maps, list(range(NCORES)))
    LAST_RESULTS = res
    Wo32 = np.asarray(Wo, dtype=np.float32)
    extra = (Wo32 @ np.asarray(bv, dtype=np.float32)
             + np.asarray(bo, dtype=np.float32))
    out = np.stack([res.results[2 * b]["y"].astype(np.float32)
                    + res.results[2 * b + 1]["y"].astype(np.float32)
                    + extra[None, :] for b in range(B)])
    return out.astype(np.float32)


# revision 36
# speedup vs baseline: 1.0384x; 1.0384x over previous
"""Multi-head attention (B=4, S=1500, D=1024, H=16) on 8 TRN2 NeuronCores.

Sharding: (batch, head-half) -> core c = 2*b + h; each core computes the full
attention for batch b, heads h*8..h*8+7, plus its partial contribution to the
output projection (contraction over its 512 features). Host sums the two
partials per batch, adds bo + Wo@bv (the v-bias term commutes through the
output projection), and stacks.

Numerics (end-to-end max-rel err ~1.5e-2, dominated by one fp8 cast of q):
  q/k/v projections run on the PE in fp8e4 + DoubleRow perf mode (0.5
  cycles/row, 256 contraction rows/instr) using a 3-term residual split
  x@W ~= x8@W8 + x8@Wr + xr@W8 (x8/W8 fp8 casts, xr/Wr fp8 casts of the
  cast residuals; the dropped xr@Wr term is ~0.07%). Wq/Wk/Wv are
  host-scaled by 32 so their fp8 mantissas are in range; the scale is
  compensated in the exp scale (q and k both carry 32x) and in the v
  staging (x 1/32).
  scores S^T[k,q] per head: one DoubleRow matmul with lhsT groups =
  (k_hi, k_lo) fp8 (exact k) and rhs groups = (q8, q8) (one fp8 cast of q:
  the only significant error). exp on ACT (the critical engine, ~138us)
  -> P^T in fp16.
  U: WVn[q, 65] += pt[k, qchunk].T @ v_aug[k, 65] in fp16 (full output
  lanes, N=65). The 12 q-tile accumulators pack into 2 PSUM banks via DVE
  memset + start=False accumulation. v's 65th ones-column gives softmax
  denominators. normalize: per-partition reciprocal + TSP -> WV[q,f] fp16;
  WV -> wvT via XBAR DMA-transpose (no PE/DVE cost). out-projection fp16.

Emission is software-pipelined around the ACT exp cadence: head-pair tensor
t0 is projected up front, t1-t3 stream through spare slots of the first
windows; v-projection borrows the uacc PSUM banks during head 0's window;
U(h) runs during head h+1's window; PSUM = 2x [128,1536] (S^T + projection
accumulators, rotating) + 2x [128,512] accumulator banks.
"""

import os
import numpy as np

N_STATE = 1024
B = 4
S = 1500
F = 512            # features per core (8 heads x 64)
NKT = 12           # seq k-tiles of 128, last = 92
KPAD = 1536
VBLK = 520         # 8 heads * 65 cols (64 d + ones) per seq tile in v_sb
SLOTW = 80         # uacc accumulator slot stride (f32 cols)
QCH = [(0, 512), (512, 512), (1024, 476)]
SCALE = 0.125      # 1/sqrt(64)
WSCALE = 32.0      # host scaling of Wq/Wk/Wv before fp8 cast
EXP_SCALE = SCALE / (WSCALE * WSCALE)
NCORES = 8

_CACHE = {}
LAST_RESULTS = None


def _build():
    import concourse.mybir as mybir
    import concourse.tile as tile
    from concourse import bacc

    f32 = mybir.dt.float32
    fp16 = mybir.dt.float16
    fp8 = mybir.dt.float8e4
    Exp = mybir.ActivationFunctionType.Exp
    Copy = mybir.ActivationFunctionType.Copy
    DR = mybir.MatmulPerfMode.DoubleRow

    nc = bacc.Bacc("TRN2", target_bir_lowering=False, debug=False,
                   num_devices=NCORES)

    x8d = nc.dram_tensor("x8d", [128, 4, 2, KPAD], fp8,
                         kind="ExternalInput").ap()
    xrd = nc.dram_tensor("xrd", [128, 4, 2, KPAD], fp8,
                         kind="ExternalInput").ap()
    wq8 = nc.dram_tensor("wq8", [4, 128, 4, 2, 128], fp8,
                         kind="ExternalInput").ap()
    wqr = nc.dram_tensor("wqr", [4, 128, 4, 2, 128], fp8,
                         kind="ExternalInput").ap()
    wk8 = nc.dram_tensor("wk8", [4, 128, 4, 2, 128], fp8,
                         kind="ExternalInput").ap()
    wkr = nc.dram_tensor("wkr", [4, 128, 4, 2, 128], fp8,
                         kind="ExternalInput").ap()
    wv8 = nc.dram_tensor("wv8", [128, 4, 2, F], fp8, kind="ExternalInput").ap()
    wvr = nc.dram_tensor("wvr", [128, 4, 2, F], fp8, kind="ExternalInput").ap()
    wod = nc.dram_tensor("wod", [128, 4, N_STATE], fp16,
                         kind="ExternalInput").ap()
    bqh = nc.dram_tensor("bqh", [128, 4], f32, kind="ExternalInput").ap()
    y = nc.dram_tensor("y", [S, N_STATE], fp16, kind="ExternalOutput").ap()

    mm = nc.tensor.matmul

    with tile.TileContext(nc) as tc:
        with (
            tc.tile_pool(name="sb", bufs=1) as sb,
            tc.tile_pool(name="sbw", bufs=16) as sbw,
            tc.tile_pool(name="ptp", bufs=22) as ptp,
            tc.tile_pool(name="sm", bufs=3) as smp,
            tc.tile_pool(name="ysp", bufs=6) as ysp,
            tc.tile_pool(name="ps3", bufs=2, space="PSUM") as ps3,
            tc.tile_pool(name="psu", bufs=1, space="PSUM") as psu,
        ):
            # ---------------- persistent SBUF ----------------
            x8_sb = sb.tile([128, 4, 2, KPAD], fp8, name="x8_sb", tag="x8")
            xr_sb = sb.tile([128, 4, 2, KPAD], fp8, name="xr_sb", tag="xr")
            # per head-pair tensor t: partitions p = head 2t + p//64,
            # d = p%64; q groups both hold q8, k groups hold (k_hi, k_lo)
            qdr = [sb.tile([128, S], fp8, name=f"qdr{t}", tag=f"qdr{t}")
                   for t in range(4)]
            kdr = [sb.tile([128, 2, KPAD], fp8, name=f"kdr{t}", tag=f"kdr{t}")
                   for t in range(4)]
            v_sb = sb.tile([128, NKT * VBLK], fp16, name="v_sb", tag="v")
            wv8_sb = sb.tile([128, 4, 2, F], fp8, name="wv8_sb", tag="wv8")
            wvr_sb = sb.tile([128, 4, 2, F], fp8, name="wvr_sb", tag="wvr")
            wo_sb = sb.tile([128, 4, N_STATE], fp16, name="wo_sb", tag="wo")
            WV_sb = sb.tile([128, NKT, F], fp16, name="WV_sb", tag="WV")
            wvT_sb = sb.tile([128, 4, KPAD], fp16, name="wvT_sb", tag="wvT")
            bq_sb = sb.tile([128, 4], f32, name="bq_sb", tag="bq")
            zero_col = sb.tile([128, 1], f32, name="zero_col", tag="zc")
            i32_col = sb.tile([128, 1], f32, name="i32_col", tag="i32")

            # uacc banks double as the v-projection / U accumulators
            uaccA = psu.tile([128, 512], f32, name="uaccA", tag="uaccA")
            uaccB = psu.tile([128, 512], f32, name="uaccB", tag="uaccB")

            # ---------------- input DMAs ----------------
            def load_w(dram, t):
                wsl = sbw.tile([128, 4, 2, 128], fp8, name="wsl", tag="wsl")
                nc.sync.dma_start(out=wsl[:], in_=dram[t])
                return wsl

            wslq8, wslqr = [load_w(wq8, 0)], [load_w(wqr, 0)]
            nc.sync.dma_start(out=x8_sb[:, :, :, 0:512],
                              in_=x8d[:, :, :, 0:512])
            nc.sync.dma_start(out=xr_sb[:, :, :, 0:512],
                              in_=xrd[:, :, :, 0:512])
            wslk8, wslkr = [load_w(wk8, 0)], [load_w(wkr, 0)]
            nc.sync.dma_start(out=bq_sb[:], in_=bqh)
            nc.vector.memset(zero_col[:], 0.0)
            nc.vector.memset(i32_col[:], 1.0 / WSCALE)
            nc.vector.memset(WV_sb[64:128, NKT - 1, :], 0.0)
            # v ones-columns and k pad columns via memset (no DMA)
            nc.vector.memset(
                v_sb[:].rearrange("p (t h c) -> p t h c",
                                  t=NKT, h=8)[:, :, :, 64:65], 1.0)
            for t in range(4):
                nc.vector.memset(kdr[t][:, :, S:KPAD], 0.0)
            # warm the ACT exp table off the critical path
            warm = smp.tile([128, 1], f32, name="warm", tag="warm")
            nc.scalar.activation(warm[:], zero_col[:], Exp, scale=1.0)
            nc.sync.dma_start(out=x8_sb[:, :, :, 512:KPAD],
                              in_=x8d[:, :, :, 512:KPAD])
            nc.sync.dma_start(out=xr_sb[:, :, :, 512:KPAD],
                              in_=xrd[:, :, :, 512:KPAD])
            nc.sync.dma_start(out=wv8_sb[:], in_=wv8)
            nc.sync.dma_start(out=wvr_sb[:], in_=wvr)
            # head-pair tensors t1-3: one DMA per weight tensor
            def load_w3(dram):
                w3 = sbw.tile([128, 3, 4, 2, 128], fp8, name="w3", tag="w3",
                              bufs=4)
                nc.sync.dma_start(
                    out=w3[:], in_=dram[1:4].rearrange("t p g i m -> p t g i m"))
                return [w3[:, t - 1] for t in range(1, 4)]
            wslq8 += load_w3(wq8)
            wslqr += load_w3(wqr)
            wslk8 += load_w3(wk8)
            wslkr += load_w3(wkr)
            nc.sync.dma_start(out=wo_sb[:], in_=wod)

            # ---------------- projections (fp8 DR, 3-term residual) -------
            def proj_chunk(w8sl, wrsl, pacc, q0, qn):
                pairs = [(w8sl, x8_sb), (wrsl, x8_sb), (w8sl, xr_sb)]
                for ti, (wsl, xs) in enumerate(pairs):
                    for g in range(4):
                        mm(out=pacc[:, q0:q0 + qn],
                           lhsT=wsl[:, g, :, :],
                           rhs=xs[:, g, :, q0:q0 + qn],
                           start=(ti == 0 and g == 0),
                           stop=(ti == 2 and g == 3), perf_mode=DR)

            def q_stage(pacc, t, q0, qn):
                nc.vector.tensor_scalar_add(
                    out=qdr[t][:, q0:q0 + qn], in0=pacc[:, q0:q0 + qn],
                    scalar1=bq_sb[:, t:t + 1])

            def k_stage(pacc, t, q0, qn):
                nc.vector.tensor_scalar_add(
                    out=kdr[t][:, 0, q0:q0 + qn], in0=pacc[:, q0:q0 + qn],
                    scalar1=zero_col[:, 0:1])
                nc.vector.tensor_sub(kdr[t][:, 1, q0:q0 + qn],
                                     pacc[:, q0:q0 + qn],
                                     kdr[t][:, 0, q0:q0 + qn])

            # one q-chunk per unit so each borrows a big3 slot only briefly
            # (allocate, accumulate 12 DR matmuls, stage)
            def late_proj_unit(qk, t, ch):
                pacc = ps3.tile([128, 1536], f32, name="pacc", tag="big3")
                q0, qn = QCH[ch]
                if qk == "q":
                    proj_chunk(wslq8[t], wslqr[t], pacc, q0, qn)
                    q_stage(pacc, t, q0, qn)
                else:
                    proj_chunk(wslk8[t], wslkr[t], pacc, q0, qn)
                    k_stage(pacc, t, q0, qn)

            # head-pair t0 up front: all q chunks + k chunk 0 (enough for
            # S^T(h0, kt0-3)); k chunks 1/2 follow as window-0 fillers
            late_proj_unit("q", 0, 0)
            late_proj_unit("k", 0, 0)
            late_proj_unit("q", 0, 1)
            late_proj_unit("q", 0, 2)

            # ---------------- attention building blocks ----------------
            def st_exp(h, kt):
                t, r = h // 2, 64 * (h % 2)
                stp = ps3.tile([128, 1536], f32, name="stp", tag="big3")
                for q0, qn in QCH:
                    mm(out=stp[:, q0:q0 + qn],
                       lhsT=kdr[t][r:r + 64, :, kt * 128:(kt + 1) * 128],
                       rhs=qdr[t][r:r + 64, q0:q0 + qn][:, None, :
                                                        ].to_broadcast(
                           (64, 2, qn)),
                       start=True, stop=True, perf_mode=DR)
                pt = ptp.tile([128, 1536], fp16, name="pt", tag="pt")
                nc.scalar.activation(pt[:, 0:S], stp[:, 0:S], Exp,
                                     scale=EXP_SCALE)
                return pt

            def u_emit(h, kt, pt, accA=None, accB=None):
                accA = uaccA if accA is None else accA
                accB = uaccB if accB is None else accB
                kn = min(128, S - kt * 128)
                vcol = kt * VBLK + h * 65
                for qt in range(NKT):
                    qn = min(128, S - qt * 128)
                    acc = accA if qt < 6 else accB
                    slot = (qt % 6) * SLOTW
                    mm(out=acc[0:qn, slot:slot + 65],
                       lhsT=pt[0:kn, qt * 128:qt * 128 + qn],
                       rhs=v_sb[0:kn, vcol:vcol + 65],
                       start=False, stop=False, skip_group_check=True)

            def v_emit(sq):
                sn = min(128, S - sq * 128)
                acc = uaccA if sq % 2 == 0 else uaccB
                first, last = True, False
                for ti, (xs, ws) in enumerate(
                        [(x8_sb, wv8_sb), (x8_sb, wvr_sb), (xr_sb, wv8_sb)]):
                    for g in range(4):
                        mm(out=acc[0:sn, 0:512],
                           lhsT=xs[:, g, :, sq * 128:sq * 128 + sn],
                           rhs=ws[:, g, :, :],
                           start=(ti == 0 and g == 0),
                           stop=(ti == 2 and g == 3), perf_mode=DR)
                nc.vector.tensor_scalar_mul(
                    out=v_sb[0:sn, sq * VBLK:(sq + 1) * VBLK].rearrange(
                        "p (h c) -> p h c", h=8)[:, :, 0:64],
                    in0=acc[0:sn, 0:512].rearrange("p (h c) -> p h c", h=8),
                    scalar1=i32_col[0:sn, 0:1])

            def memsets():
                nc.vector.memset(uaccA[:], 0.0)
                nc.vector.memset(uaccB[:], 0.0)

            def norm_emit(h, accA=None, accB=None):
                accA = uaccA if accA is None else accA
                accB = uaccB if accB is None else accB
                rc = smp.tile([128, 12], f32, name="rc", tag="rc")
                nc.vector.reciprocal(
                    rc[:, 0:6].rearrange("p (s c) -> p s c", c=1),
                    accA[:, 0:6 * SLOTW].rearrange(
                        "p (s c) -> p s c", s=6)[:, :, 64:65])
                nc.vector.reciprocal(
                    rc[:, 6:11].rearrange("p (s c) -> p s c", c=1),
                    accB[:, 0:5 * SLOTW].rearrange(
                        "p (s c) -> p s c", s=5)[:, :, 64:65])
                nc.vector.reciprocal(rc[0:92, 11:12],
                                     accB[0:92, 5 * SLOTW + 64:5 * SLOTW + 65])
                for qt in range(NKT):
                    sn = min(128, S - qt * 128)
                    acc = accA if qt < 6 else accB
                    slot = (qt % 6) * SLOTW
                    nc.vector.tensor_scalar_mul(
                        out=WV_sb[0:sn, qt, h * 64:(h + 1) * 64],
                        in0=acc[0:sn, slot:slot + 64],
                        scalar1=rc[0:sn, qt:qt + 1])

            def transp_emit(p):
                for qt in range(NKT):
                    nc.sync.dma_start(
                        out=wvT_sb[:, p, qt * 128:(qt + 1) * 128],
                        in_=WV_sb[:, qt, p * 128:(p + 1) * 128],
                        transpose=True)

            # one q-chunk per unit so each borrows a big3 slot only briefly
            # (allocate, accumulate 12 DR matmuls, stage)
            def late_proj_unit(qk, t, ch):
                pacc = ps3.tile([128, 1536], f32, name="pacc", tag="big3")
                q0, qn = QCH[ch]
                if qk == "q":
                    proj_chunk(wslq8[t], wslqr[t], pacc, q0, qn)
                    q_stage(pacc, t, q0, qn)
                else:
                    proj_chunk(wslk8[t], wslkr[t], pacc, q0, qn)
                    k_stage(pacc, t, q0, qn)

            # head-pair t0 up front: all q chunks + k chunk 0 (enough for
            # S^T(h0, kt0-3)); k chunks 1/2 follow as window-0 fillers
            late_proj_unit("q", 0, 0)
            late_proj_unit("k", 0, 0)
            late_proj_unit("q", 0, 1)
            late_proj_unit("q", 0, 2)

            # ---------------- attention building blocks ----------------
            def st_exp(h, kt):
                t, r = h // 2, 64 * (h % 2)
                stp = ps3.tile([128, 1536], f32, name="stp", tag="big3")
                for q0, qn in QCH:
                    mm(out=stp[:, q0:q0 + qn],
                       lhsT=kdr[t][r:r + 64, :, kt * 128:(kt + 1) * 128],
                       rhs=qdr[t][r:r + 64, q0:q0 + qn][:, None, :
                                                        ].to_broadcast(
                           (64, 2, qn)),
                       start=True, stop=True, perf_mode=DR)
                pt = ptp.tile([128, 1536], fp16, name="pt", tag="pt")
                nc.scalar.activation(pt[:, 0:S], stp[:, 0:S], Exp,
                                     scale=EXP_SCALE)
                return pt

            def u_emit(h, kt, pt, accA=None, accB=None):
                accA = uaccA if accA is None else accA
                accB = uaccB if accB is None else accB
                kn = min(128, S - kt * 128)
                vcol = kt * VBLK + h * 65
                for qt in range(NKT):
                    qn = min(128, S - qt * 128)
                    acc = accA if qt < 6 else accB
                    slot = (qt % 6) * SLOTW
                    mm(out=acc[0:qn, slot:slot + 65],
                       lhsT=pt[0:kn, qt * 128:qt * 128 + qn],
                       rhs=v_sb[0:kn, vcol:vcol + 65],
                       start=False, stop=False, skip_group_check=True)

            def v_emit(sq):
                sn = min(128, S - sq * 128)
                acc = uaccA if sq % 2 == 0 else uaccB
                first, last = True, False
                for ti, (xs, ws) in enumerate(
                        [(x8_sb, wv8_sb), (x8_sb, wvr_sb), (xr_sb, wv8_sb)]):
                    for g in range(4):
                        mm(out=acc[0:sn, 0:512],
                           lhsT=xs[:, g, :, sq * 128:sq * 128 + sn],
                           rhs=ws[:, g, :, :],
                           start=(ti == 0 and g == 0),
                           stop=(ti == 2 and g == 3), perf_mode=DR)
                nc.vector.tensor_scalar_mul(
                    out=v_sb[0:sn, sq * VBLK:(sq + 1) * VBLK].rearrange(
                        "p (h c) -> p h c", h=8)[:, :, 0:64],
                    in0=acc[0:sn, 0:512].rearrange("p (h c) -> p h c", h=8),
                    scalar1=i32_col[0:sn, 0:1])

            def memsets():
                nc.vector.memset(uaccA[:], 0.0)
                nc.vector.memset(uaccB[:], 0.0)

            def norm_emit(h, accA=None, accB=None):
                accA = uaccA if accA is None else accA
                accB = uaccB if accB is None else accB
                rc = smp.tile([128, 12], f32, name="rc", tag="rc")
                nc.vector.reciprocal(
                    rc[:, 0:6].rearrange("p (s c) -> p s c", c=1),
                    accA[:, 0:6 * SLOTW].rearrange(
                        "p (s c) -> p s c", s=6)[:, :, 64:65])
                nc.vector.reciprocal(
                    rc[:, 6:11].rearrange("p (s c) -> p s c", c=1),
                    accB[:, 0:5 * SLOTW].rearrange(
                        "p (s c) -> p s c", s=5)[:, :, 64:65])
                nc.vector.reciprocal(rc[0:92, 11:12],
                                     accB[0:92, 5 * SLOTW + 64:5 * SLOTW + 65])
                for qt in range(NKT):
                    sn = min(128, S - qt * 128)
                    acc = accA if qt < 6 else accB
                    slot = (qt % 6) * SLOTW
                    nc.vector.tensor_scalar_mul(
                        out=WV_sb[0:sn, qt, h * 64:(h + 1) * 64],
                        in0=acc[0:sn, slot:slot + 64],
                        scalar1=rc[0:sn, qt:qt + 1])

            def transp_emit(p):
                for qt in range(NKT):
                    nc.sync.dma_start(
                        out=wvT_sb[:, p, qt * 128:(qt + 1) * 128],
                        in_=WV_sb[:, qt, p * 128:(p + 1) * 128],
                        transpose=True)

            # out-projection split in two parts: fc0-2 accumulate early, fc3
            # joins once the pair-3 transposes land; both halves of a q-tile
            # stage into one yt so y goes out as a single DMA per q-tile
            def out_a(sq, ch):
                sn = min(128, S - sq * 128)
                py = ps3.tile([128, 1536], f32, name="py", tag="big3")
                for fc in range(3):
                    mm(out=py[0:sn, 0:512],
                       lhsT=wvT_sb[:, fc, sq * 128:sq * 128 + sn],
                       rhs=wo_sb[:, fc, ch * 512:(ch + 1) * 512],
                       start=(fc == 0), stop=False, skip_group_check=True)
                return py

            def out_b(sq, ch, py, yt):
                sn = min(128, S - sq * 128)
                mm(out=py[0:sn, 0:512],
                   lhsT=wvT_sb[:, 3, sq * 128:sq * 128 + sn],
                   rhs=wo_sb[:, 3, ch * 512:(ch + 1) * 512],
                   start=False, stop=True, skip_group_check=True)
                nc.vector.tensor_scalar_add(out=yt[0:sn, ch * 512:(ch + 1) * 512],
                                            in0=py[0:sn, 0:512],
                                            scalar1=zero_col[0:sn, 0:1])
                if ch == 1:
                    nc.sync.dma_start(out=y[sq * 128:sq * 128 + sn, :],
                                      in_=yt[0:sn, :])

            # ---------------- pipelined attention ----------------
            # window h, slot kt: S^T/exp(h, kt), then the scheduled fillers
            FILL = {}

            def add(h, kt, fn):
                FILL.setdefault((h, kt), []).append(fn)

            add(0, 0, lambda: late_proj_unit("k", 0, 1))
            add(0, 1, lambda: late_proj_unit("k", 0, 2))
            for kt in range(NKT):
                add(0, kt, lambda sq=kt: v_emit(sq))
            add(1, 0, memsets)
            for kt in range(1, NKT):
                add(1, kt, lambda k=kt - 1: u_emit(0, k, pts.pop((0, k))))
            add(1, 11, lambda: u_emit(0, 11, pts.pop((0, 11))))
            # t1 projections during window 1 (needed by window 2)
            for ci, kt in enumerate((2, 4, 6)):
                add(1, kt, lambda c=ci: late_proj_unit("q", 1, c))
            for ci, kt in enumerate((3, 5, 7)):
                add(1, kt, lambda c=ci: late_proj_unit("k", 1, c))
            for h in range(2, 8):
                add(h, 0, lambda hh=h - 2: norm_emit(hh))
                add(h, 1, memsets)
                add(h, 1, lambda hh=h - 1: u_emit(hh, 0, pts.pop((hh, 0))))
                for kt in range(2, NKT):
                    add(h, kt,
                        lambda hh=h - 1, k=kt - 1: u_emit(hh, k,
                                                          pts.pop((hh, k))))
                add(h, 11, lambda hh=h - 1: u_emit(hh, 11,
                                                   pts.pop((hh, 11))))
            # t2 over windows 2-3, t3 over windows 4-5 (first uses: w4 / w6)
            for ci, (hh, kt) in enumerate(((2, 3), (2, 6), (2, 9))):
                add(hh, kt, lambda c=ci: late_proj_unit("q", 2, c))
            for ci, (hh, kt) in enumerate(((3, 3), (3, 6), (3, 9))):
                add(hh, kt, lambda c=ci: late_proj_unit("k", 2, c))
            for ci, (hh, kt) in enumerate(((4, 3), (4, 6), (4, 9))):
                add(hh, kt, lambda c=ci: late_proj_unit("q", 3, c))
            for ci, (hh, kt) in enumerate(((5, 3), (5, 6), (5, 9))):
                add(hh, kt, lambda c=ci: late_proj_unit("k", 3, c))
            add(4, 1, lambda: transp_emit(0))
            add(6, 1, lambda: transp_emit(1))
            add(7, 1, lambda: transp_emit(2))

            pts = {}
            for h in range(8):
                for kt in range(NKT):
                    pts[(h, kt)] = st_exp(h, kt)
                    for fn in FILL.get((h, kt), ()):
                        fn()

            # ---------------- tail ----------------
            # head 7's U accumulates into a spare big3 slot (two clean
            # 512-col bank regions) so norm(6) can drain the uacc banks in
            # parallel; U(7) runs qt-outer with per-qt normalize + transpose
            # so the pair-3 transposes trickle out as early as possible
            u2 = ps3.tile([128, 1536], f32, name="u2", tag="big3")
            nc.vector.memset(u2[:, 0:1024], 0.0)
            u2A, u2B = u2[:, 0:512], u2[:, 512:1024]
            norm_emit(6)
            rc7 = smp.tile([128, 12], f32, name="rc7", tag="rc")
            for qt in range(NKT):
                qn = min(128, S - qt * 128)
                acc = u2A if qt < 6 else u2B
                slot = (qt % 6) * SLOTW
                for kt in range(NKT):
                    kn = min(128, S - kt * 128)
                    pt = pts[(7, kt)]
                    mm(out=acc[0:qn, slot:slot + 65],
                       lhsT=pt[0:kn, qt * 128:qt * 128 + qn],
                       rhs=v_sb[0:kn, kt * VBLK + 7 * 65:kt * VBLK + 7 * 65 + 65],
                       start=False, stop=False, skip_group_check=True)
                nc.vector.reciprocal(rc7[0:qn, qt:qt + 1],
                                     acc[0:qn, slot + 64:slot + 65])
                nc.vector.tensor_scalar_mul(
                    out=WV_sb[0:qn, qt, 7 * 64:8 * 64],
                    in0=acc[0:qn, slot:slot + 64],
                    scalar1=rc7[0:qn, qt:qt + 1])
                nc.sync.dma_start(
                    out=wvT_sb[:, 3, qt * 128:(qt + 1) * 128],
                    in_=WV_sb[:, qt, 384:512],
                    transpose=True)
            for kt in range(NKT):
                pts.pop((7, kt))
            # out-projection with a 4-deep psum rotation (2 big3 slots +
            # the two freed uacc banks); fc0-2 accumulate first, fc3 joins
            # when the pair-3 transpose for that q-tile lands
            seq = [(sq, ch) for sq in range(NKT) for ch in range(2)]
            pys = {}
            yts = {}
            LAG = 3

            def out_fin(s2, c2):
                sn2 = min(128, S - s2 * 128)
                reg = pys.pop((s2, c2))
                mm(out=reg[0:sn2, 0:512],
                   lhsT=wvT_sb[:, 3, s2 * 128:s2 * 128 + sn2],
                   rhs=wo_sb[:, 3, c2 * 512:(c2 + 1) * 512],
                   start=False, stop=True, skip_group_check=True)
                nc.vector.tensor_scalar_add(
                    out=yts[s2][0:sn2, c2 * 512:(c2 + 1) * 512],
                    in0=reg[0:sn2, 0:512], scalar1=zero_col[0:sn2, 0:1])
                if c2 == 1:
                    nc.sync.dma_start(out=y[s2 * 128:s2 * 128 + sn2, :],
                                      in_=yts[s2][0:sn2, :])

            for i, (sq, ch) in enumerate(seq):
                yts.setdefault(sq, ysp.tile([128, 1024], fp16, name="yt",
                                            tag="yt"))
                sn = min(128, S - sq * 128)
                if i % 4 < 2:
                    reg = ps3.tile([128, 1536], f32, name="py", tag="big3")
                else:
                    reg = uaccA if i % 4 == 2 else uaccB
                for fc in range(3):
                    mm(out=reg[0:sn, 0:512],
                       lhsT=wvT_sb[:, fc, sq * 128:sq * 128 + sn],
                       rhs=wo_sb[:, fc, ch * 512:(ch + 1) * 512],
                       start=(fc == 0), stop=False, skip_group_check=True)
                pys[(sq, ch)] = reg
                if i >= LAG:
                    out_fin(*seq[i - LAG])
            for j in range(len(seq) - LAG, len(seq)):
                out_fin(*seq[j])

    nc.compile()
    return nc


def get_nc():
    if "nc" not in _CACHE:
        _CACHE["nc"] = _build()
    return _CACHE["nc"]


def make_in_maps(x, Wq, bq, Wk, Wv, bv, Wo, bo):
    import ml_dtypes
    e4 = ml_dtypes.float8_e4m3fn
    f16 = np.float16

    x = np.asarray(x, dtype=np.float32)
    Wq = np.asarray(Wq, dtype=np.float32)
    Wk = np.asarray(Wk, dtype=np.float32)
    Wv = np.asarray(Wv, dtype=np.float32)
    Wo = np.asarray(Wo, dtype=np.float32)
    bq = np.asarray(bq, dtype=np.float32)

    def wsplit(Wc):
        # Wc: [512 feat, 1024 state] scaled; -> (hi, lo) each
        # [4, 128(p), 4(g), 2(i), 128(m)] fp8 with W[128t+m, (2g+i)*128+p]
        W8 = Wc.astype(e4)
        Wr = (Wc - W8.astype(np.float32)).astype(e4)
        out = []
        for Wx in (W8, Wr):
            a = Wx.T.reshape(4, 2, 128, 4, 128)     # [g, i, p, t, m]
            out.append(np.ascontiguousarray(a.transpose(3, 2, 0, 1, 4)))
        return out

    def vsplit(Wc):
        # -> [128(p), 4(g), 2(i), 512(f)] fp8
        W8 = Wc.astype(e4)
        Wr = (Wc - W8.astype(np.float32)).astype(e4)
        out = []
        for Wx in (W8, Wr):
            a = Wx.T.reshape(4, 2, 128, F)          # [g, i, p, f]
            out.append(np.ascontiguousarray(a.transpose(2, 0, 1, 3)))
        return out

    in_maps = []
    for c in range(NCORES):
        b, h2 = divmod(c, 2)
        sl = slice(h2 * F, (h2 + 1) * F)
        xT = np.zeros((N_STATE, KPAD), dtype=np.float32)
        xT[:, 0:S] = x[b].T
        x8f = xT.astype(e4)
        xrf = (xT - x8f.astype(np.float32)).astype(e4)
        x8 = x8f.reshape(4, 2, 128, KPAD).transpose(2, 0, 1, 3)
        xr = xrf.reshape(4, 2, 128, KPAD).transpose(2, 0, 1, 3)
        wq8_, wqr_ = wsplit(Wq[sl] * WSCALE)
        wk8_, wkr_ = wsplit(Wk[sl] * WSCALE)
        wv8_, wvr_ = vsplit(Wv[sl] * WSCALE)
        wod_ = Wo[:, sl].T.reshape(4, 128, N_STATE).transpose(1, 0, 2)
        # feature f = 128t + p  ->  bqh[p, t]
        bqh_ = (WSCALE * bq[sl]).reshape(4, 128).T
        in_maps.append(dict(
            x8d=np.ascontiguousarray(x8), xrd=np.ascontiguousarray(xr),
            wq8=wq8_, wqr=wqr_, wk8=wk8_, wkr=wkr_,
            wv8=wv8_, wvr=wvr_,
            wod=np.ascontiguousarray(wod_.astype(f16)),
            bqh=np.ascontiguousarray(bqh_, dtype=np.float32),
        ))
    return in_maps


def kernel(x, Wq, bq, Wk, Wv, bv, Wo, bo):
    global LAST_RESULTS
    from concourse.bass_utils import run_bass_kernel_spmd

    try:
        import antenv.axon_hooks  # noqa: F401
    except ImportError:
        os.environ["BASS_NEVER_TRACE"] = "1"

    nc = get_nc()
    in_maps = make_in_maps(x, Wq, bq, Wk, Wv, bv, Wo, bo)
    res = run_bass_kernel_spmd(nc, in_maps, list(range(NCORES)))
    LAST_RESULTS = res
    Wo32 = np.asarray(Wo, dtype=np.float32)
    extra = (Wo32 @ np.asarray(bv, dtype=np.float32)
             + np.asarray(bo, dtype=np.float32))
    out = np.stack([res.results[2 * b]["y"].astype(np.float32)
                    + res.results[2 * b + 1]["y"].astype(np.float32)
                    + extra[None, :] for b in range(B)])
    return out.astype(np.float32)
